# revision 27
# baseline (speedup 1.0000x reference)
"""VQ codebook kernel for Trainium2 (Bass/Tile), 8-core data-parallel.

Computes, for batch [64,32,32,64] and codebook embeddings [64,512]:
  flat = batch.reshape(-1, 64)                       # [N=65536, 64]
  dist = ||flat||^2 + ||emb||^2 - 2 flat@emb         # [N, 512]
  idx  = argmin(dist, axis=1)
  enc  = one_hot(idx, 512) (f32)                     # [N, 512]
  outv = emb[:, idx].T reshaped to batch shape       # [64,32,32,64]
returns (enc, outv) like the reference.

Per core (N sharded 8192 rows/core, 64 tiles of 128 rows):
  - 2sim = 2*X@emb computed to ~fp32 accuracy as three accumulated bf16
    matmuls over hi/mid/lo splits (X=Xhi+Xmid+Xlo, 2emb=Ehi+Emid+Elo):
      MM_A: [XhiT;XmidT] . [Ehi;Ehi]   -> hh + mh
      MM_B: [XhiT;XmidT] . [Emid;Emid] -> hm + mm   (same weights as A)
      MM_C: [XhiT;XloT]  . [Elo;Ehi]   -> hl + lh
    lhsT tiles are host-pre-transposed bf16 stacks, DMA'd directly.
  - ACT: t1 = fp32(e2 + f2); DVE: dist = fl(t1 - 2sim) — reproducing the
    reference's exact rounding structure fl(fl(f2+e2) - 2sim) so
    near-ties land on the same fp32 grid jnp's argmin sees (0 argmin
    mismatches measured on hardware).
  - DVE: reduce_min + max_index (value match) -> first-index argmin.
  - enc: dense one-hot built on ACT as (sign(min - dist) + 1) — exact
    0.0/1.0 floats (the reference dist has no duplicated minima, so
    value-equality equals one_hot) — streamed out as 256KB tiles.
  - outv: per-tile indirect DMA gather of emb^T rows (hardware supports
    one offset per partition per indirect DMA).
"""

import sys
import numpy as np

sys.path.insert(0, "/opt/trn_rl_repo")

from contextlib import ExitStack  # noqa: E402

import ml_dtypes  # noqa: E402

import concourse.bacc as bacc  # noqa: E402
import concourse.mybir as mybir  # noqa: E402
import concourse.tile as tile  # noqa: E402
from concourse.bass import IndirectOffsetOnAxis  # noqa: E402
from concourse.bass_utils import run_bass_kernel_spmd  # noqa: E402

B, H, W, D, K = 64, 32, 32, 64, 512
N = B * H * W              # 65536 rows total
NCORES = 8
RPC = N // NCORES          # 8192 rows per core
P = 128                    # partitions / rows per tile
NTILES = RPC // P          # 64 tiles per core

f32 = mybir.dt.float32
bf16 = mybir.dt.bfloat16
u32 = mybir.dt.uint32
npbf16 = ml_dtypes.bfloat16

_NC = None  # cached Bass program


def _build_nc():
    # Bacc (not raw Bass): its compile() splits sync waits to satisfy the
    # per-instruction wait-slot limits (S3_LW / DMA structs take 1 wait).
    nc = bacc.Bacc("TRN2", target_bir_lowering=False, debug=False, num_devices=NCORES)

    # host-pre-transposed bf16 lhsT stacks
    s1 = nc.declare_dram_parameter("s1", [P, RPC], bf16, isOutput=False)  # [XhiT;XmidT]
    s3 = nc.declare_dram_parameter("s3", [P, RPC], bf16, isOutput=False)  # [XhiT;XloT]
    ra = nc.declare_dram_parameter("ra", [P, K], bf16, isOutput=False)    # [Ehi;Ehi]
    rb = nc.declare_dram_parameter("rb", [P, K], bf16, isOutput=False)    # [Emid;Emid]
    rc = nc.declare_dram_parameter("rc", [P, K], bf16, isOutput=False)    # [Elo;Ehi]
    e2bc = nc.declare_dram_parameter("e2bc", [P, K], f32, isOutput=False)
    f2a = nc.declare_dram_parameter("f2a", [P, NTILES], f32, isOutput=False)
    embt = nc.declare_dram_parameter("embt", [K, D], f32, isOutput=False)

    enc = nc.declare_dram_parameter("enc", [RPC, K], f32, isOutput=True)
    outv = nc.declare_dram_parameter("outv", [RPC, D], f32, isOutput=True)

    with tile.TileContext(nc) as tc:
        with ExitStack() as ctx:
            cpool = ctx.enter_context(tc.tile_pool(name="consts", bufs=1))
            spool = ctx.enter_context(tc.tile_pool(name="sbuf", bufs=4))
            mpool = ctx.enter_context(
                tc.tile_pool(name="mpsum", bufs=4, space="PSUM")
            )

            ra_sb = cpool.tile([P, K], bf16)
            nc.sync.dma_start(ra_sb[:], ra[:])
            rb_sb = cpool.tile([P, K], bf16)
            nc.sync.dma_start(rb_sb[:], rb[:])
            rc_sb = cpool.tile([P, K], bf16)
            nc.sync.dma_start(rc_sb[:], rc[:])
            e2_sb = cpool.tile([P, K], f32)
            nc.sync.dma_start(e2_sb[:], e2bc[:])
            f2_sb = cpool.tile([P, NTILES], f32)
            nc.sync.dma_start(f2_sb[:], f2a[:])

            # PE warmups: absorb const-DMA waits onto PE's vector clock so
            # per-matmul sync waits stay within the hardware wait slots.
            wpool = ctx.enter_context(
                tc.tile_pool(name="wpsum", bufs=1, space="PSUM")
            )
            for wtag, wsb in (("w1", ra_sb), ("w2", rb_sb), ("w3", rc_sb)):
                wps = wpool.tile([P, 8], f32, space="PSUM", tag=wtag)
                nc.tensor.matmul(
                    wps[:], lhsT=wsb[:, 0:P], rhs=wsb[:, 0:8],
                    start=True, stop=True,
                )

            for t in range(NTILES):
                cols = slice(t * P, (t + 1) * P)

                s1_sb = spool.tile([P, P], bf16)
                nc.sync.dma_start(s1_sb[:], s1[:, cols])
                s3_sb = spool.tile([P, P], bf16)
                nc.sync.dma_start(s3_sb[:], s3[:, cols])

                # 2sim accumulated over the three product pairs
                sim_ps = mpool.tile([P, K], f32, space="PSUM")
                nc.tensor.matmul(
                    sim_ps[:], lhsT=s1_sb[:], rhs=ra_sb[:],
                    start=True, stop=False,
                )
                nc.tensor.matmul(
                    sim_ps[:], lhsT=s1_sb[:], rhs=rb_sb[:],
                    start=False, stop=False,
                )
                nc.tensor.matmul(
                    sim_ps[:], lhsT=s3_sb[:], rhs=rc_sb[:],
                    start=False, stop=True,
                )

                # t1 = fp32(e2 + f2) — the reference's first rounding
                t1_sb = spool.tile([P, K], f32)
                nc.scalar.activation(
                    t1_sb[:],
                    e2_sb[:],
                    mybir.ActivationFunctionType.Identity,
                    bias=f2_sb[:, t : t + 1],
                    scale=1.0,
                )

                # dist = fl(t1 - 2sim) — the reference's second rounding
                dist_sb = spool.tile([P, K], f32)
                nc.vector.tensor_tensor(
                    out=dist_sb[:], in0=t1_sb[:], in1=sim_ps[:],
                    op=mybir.AluOpType.subtract,
                )
                minv = spool.tile([P, 1], f32)
                nc.vector.tensor_reduce(
                    out=minv[:], in_=dist_sb[:],
                    axis=mybir.AxisListType.X, op=mybir.AluOpType.min,
                )

                # first index where dist == min  (argmin tie-break)
                idx8 = spool.tile([P, 8], u32)
                nc.vector.max_index(
                    idx8[:], minv[:, 0:1].to_broadcast([P, 8]), dist_sb[:]
                )

                # one-hot: sign(min - dist) + 1 -> exact 1.0 at the min,
                # 0.0 elsewhere (no duplicated minima in the fp32 dist)
                sgn_sb = spool.tile([P, K], f32)
                nc.scalar.activation(
                    sgn_sb[:],
                    dist_sb[:],
                    mybir.ActivationFunctionType.Sign,
                    bias=minv[:, 0:1],
                    scale=-1.0,
                )
                enc_sb = spool.tile([P, K], f32)
                nc.gpsimd.tensor_scalar_add(enc_sb[:], sgn_sb[:], 1.0)
                nc.sync.dma_start(enc[t * P : (t + 1) * P, :], enc_sb[:])

                # gather quantized rows emb^T[idx]
                vq_sb = spool.tile([P, D], f32)
                nc.gpsimd.indirect_dma_start(
                    out=vq_sb[:],
                    out_offset=None,
                    in_=embt[:, :],
                    in_offset=IndirectOffsetOnAxis(ap=idx8[:, 0:1], axis=0),
                )
                nc.sync.dma_start(outv[t * P : (t + 1) * P, :], vq_sb[:])

    if not nc.is_finalized():
        nc.finalize()
    return nc


def _host_inputs(batch, embeddings):
    flat = np.ascontiguousarray(batch.reshape(N, D).astype(np.float32))
    emb = np.ascontiguousarray(embeddings.astype(np.float32))
    f32n = np.float32

    e2 = np.sum(emb * emb, axis=0, dtype=f32n)                  # [K]
    f2 = np.sum(flat * flat, axis=1, dtype=f32n)                # [N]

    E = (2.0 * emb).astype(f32n)
    Ehi = E.astype(npbf16)
    Er = (E - Ehi.astype(f32n)).astype(f32n)
    Emid = Er.astype(npbf16)
    Elo = (Er - Emid.astype(f32n)).astype(npbf16)

    Xhi = flat.astype(npbf16)
    Xr = (flat - Xhi.astype(f32n)).astype(f32n)
    Xmid = Xr.astype(npbf16)
    Xlo = (Xr - Xmid.astype(f32n)).astype(npbf16)

    s1 = np.concatenate([Xhi.T, Xmid.T], axis=0)                # [128, N]
    s3 = np.concatenate([Xhi.T, Xlo.T], axis=0)                 # [128, N]
    ra = np.concatenate([Ehi, Ehi], axis=0)                     # [128, 512]
    rb = np.concatenate([Emid, Emid], axis=0)                   # [128, 512]
    rc = np.concatenate([Elo, Ehi], axis=0)                     # [128, 512]

    e2bc = np.broadcast_to(e2[None, :], (P, K)).copy()          # [128, 512]
    embt = np.ascontiguousarray(emb.T)                          # [512, 64]

    in_maps = []
    for c in range(NCORES):
        sl = slice(c * RPC, (c + 1) * RPC)
        f2c = f2[sl].reshape(NTILES, P).T.astype(f32n)          # [128, 64]
        in_maps.append(
            {
                "s1": np.ascontiguousarray(s1[:, sl]),
                "s3": np.ascontiguousarray(s3[:, sl]),
                "ra": ra,
                "rb": rb,
                "rc": rc,
                "e2bc": e2bc,
                "f2a": np.ascontiguousarray(f2c),
                "embt": embt,
            }
        )
    return in_maps


def _get_nc():
    global _NC
    if _NC is None:
        _NC = _build_nc()
    return _NC


def run(batch, embeddings, trace=False):
    """Run on 8 NeuronCores. Returns ((enc, outv), BassKernelResults)."""
    nc = _get_nc()
    in_maps = _host_inputs(batch, embeddings)
    res = run_bass_kernel_spmd(nc, in_maps, list(range(NCORES)), trace=trace)
    enc = np.concatenate(
        [res.results[c]["enc"] for c in range(NCORES)], axis=0
    )
    outv = np.concatenate(
        [res.results[c]["outv"] for c in range(NCORES)], axis=0
    ).reshape(B, H, W, D)
    return (enc, outv), res


def kernel(batch, embeddings):
    (enc, outv), _ = run(batch, embeddings)
    return enc, outv


if __name__ == "__main__":
    rng = np.random.default_rng(0)
    batch = rng.standard_normal((B, H, W, D), dtype=np.float32)
    emb = rng.uniform(-0.05, 0.05, (D, K)).astype(np.float32)
    enc, outv = kernel(batch, emb)
    print(enc.shape, outv.shape, enc.sum(), np.abs(outv).mean())


# revision 29
# speedup vs baseline: 2.8565x; 2.8565x over previous
"""VQ codebook kernel for Trainium2 (Bass/Tile), 8-core data-parallel.

Computes, for batch [64,32,32,64] and codebook embeddings [64,512]:
  flat = batch.reshape(-1, 64)                       # [N=65536, 64]
  dist = ||flat||^2 + ||emb||^2 - 2 flat@emb         # [N, 512]
  idx  = argmin(dist, axis=1)
  enc  = one_hot(idx, 512) (f32)                     # [N, 512]
  outv = emb[:, idx].T reshaped to batch shape       # [64,32,32,64]
returns (enc, outv) like the reference.

Per core (N sharded 8192 rows/core, 64 tiles of 128 rows):
  - 2sim = 2*X@emb computed to ~fp32 accuracy as three accumulated bf16
    matmuls over hi/mid/lo splits (X=Xhi+Xmid+Xlo, 2emb=Ehi+Emid+Elo):
      MM_A: [XhiT;XmidT] . [Ehi;Ehi]   -> hh + mh
      MM_B: [XhiT;XmidT] . [Emid;Emid] -> hm + mm   (same weights as A)
      MM_C: [XhiT;XloT]  . [Elo;Ehi]   -> hl + lh
    lhsT tiles are host-pre-transposed bf16 stacks, DMA'd directly.
  - ACT: t1 = fp32(e2 + f2); DVE: dist = fl(t1 - 2sim) — reproducing the
    reference's exact rounding structure fl(fl(f2+e2) - 2sim) so
    near-ties land on the same fp32 grid jnp's argmin sees (0 argmin
    mismatches measured on hardware).
  - DVE: reduce_min + max_index (value match) -> first-index argmin.
  - enc: dense one-hot built on ACT as (sign(min - dist) + 1) — exact
    0.0/1.0 floats (the reference dist has no duplicated minima, so
    value-equality equals one_hot) — streamed out as 256KB tiles.
  - outv: per-tile indirect DMA gather of emb^T rows (hardware supports
    one offset per partition per indirect DMA).
"""

import sys
import numpy as np

sys.path.insert(0, "/opt/trn_rl_repo")

from contextlib import ExitStack  # noqa: E402

import ml_dtypes  # noqa: E402

import concourse.bacc as bacc  # noqa: E402
import concourse.mybir as mybir  # noqa: E402
import concourse.tile as tile  # noqa: E402
from concourse.bass import IndirectOffsetOnAxis  # noqa: E402
from concourse.bass_utils import run_bass_kernel_spmd  # noqa: E402

B, H, W, D, K = 64, 32, 32, 64, 512
N = B * H * W              # 65536 rows total
NCORES = 8
RPC = N // NCORES          # 8192 rows per core
P = 128                    # partitions / rows per tile
NTILES = RPC // P          # 64 tiles per core

f32 = mybir.dt.float32
bf16 = mybir.dt.bfloat16
u32 = mybir.dt.uint32
npbf16 = ml_dtypes.bfloat16

_NC = None  # cached Bass program


def _build_nc():
    # Bacc (not raw Bass): its compile() splits sync waits to satisfy the
    # per-instruction wait-slot limits (S3_LW / DMA structs take 1 wait).
    nc = bacc.Bacc("TRN2", target_bir_lowering=False, debug=False, num_devices=NCORES)

    # host-pre-transposed bf16 lhsT stacks
    s1 = nc.declare_dram_parameter("s1", [P, RPC], bf16, isOutput=False)  # [XhiT;XmidT]
    s3 = nc.declare_dram_parameter("s3", [P, RPC], bf16, isOutput=False)  # [XhiT;XloT]
    ra = nc.declare_dram_parameter("ra", [P, K], bf16, isOutput=False)    # [Ehi;Ehi]
    rb = nc.declare_dram_parameter("rb", [P, K], bf16, isOutput=False)    # [Emid;Emid]
    rc = nc.declare_dram_parameter("rc", [P, K], bf16, isOutput=False)    # [Elo;Ehi]
    e2bc = nc.declare_dram_parameter("e2bc", [P, K], f32, isOutput=False)
    f2a = nc.declare_dram_parameter("f2a", [P, NTILES], f32, isOutput=False)
    embt = nc.declare_dram_parameter("embt", [K, D], f32, isOutput=False)

    enc = nc.declare_dram_parameter("enc", [RPC, K], f32, isOutput=True)
    outv = nc.declare_dram_parameter("outv", [RPC, D], f32, isOutput=True)

    with tile.TileContext(nc) as tc:
        with ExitStack() as ctx:
            cpool = ctx.enter_context(tc.tile_pool(name="consts", bufs=1))
            spool = ctx.enter_context(tc.tile_pool(name="sbuf", bufs=4))
            mpool = ctx.enter_context(
                tc.tile_pool(name="mpsum", bufs=4, space="PSUM")
            )

            ra_sb = cpool.tile([P, K], bf16)
            nc.sync.dma_start(ra_sb[:], ra[:])
            rb_sb = cpool.tile([P, K], bf16)
            nc.sync.dma_start(rb_sb[:], rb[:])
            rc_sb = cpool.tile([P, K], bf16)
            nc.sync.dma_start(rc_sb[:], rc[:])
            e2_sb = cpool.tile([P, K], f32)
            nc.sync.dma_start(e2_sb[:], e2bc[:])
            f2_sb = cpool.tile([P, NTILES], f32)
            nc.sync.dma_start(f2_sb[:], f2a[:])

            # PE warmups: absorb const-DMA waits onto PE's vector clock so
            # per-matmul sync waits stay within the hardware wait slots.
            wpool = ctx.enter_context(
                tc.tile_pool(name="wpsum", bufs=1, space="PSUM")
            )
            for wtag, wsb in (("w1", ra_sb), ("w2", rb_sb), ("w3", rc_sb)):
                wps = wpool.tile([P, 8], f32, space="PSUM", tag=wtag)
                nc.tensor.matmul(
                    wps[:], lhsT=wsb[:, 0:P], rhs=wsb[:, 0:8],
                    start=True, stop=True,
                )

            for t in range(NTILES):
                cols = slice(t * P, (t + 1) * P)

                s1_sb = spool.tile([P, P], bf16)
                nc.sync.dma_start(s1_sb[:], s1[:, cols])
                s3_sb = spool.tile([P, P], bf16)
                nc.sync.dma_start(s3_sb[:], s3[:, cols])

                # 2sim accumulated over the three product pairs
                sim_ps = mpool.tile([P, K], f32, space="PSUM")
                nc.tensor.matmul(
                    sim_ps[:], lhsT=s1_sb[:], rhs=ra_sb[:],
                    start=True, stop=False,
                )
                nc.tensor.matmul(
                    sim_ps[:], lhsT=s1_sb[:], rhs=rb_sb[:],
                    start=False, stop=False,
                )
                nc.tensor.matmul(
                    sim_ps[:], lhsT=s3_sb[:], rhs=rc_sb[:],
                    start=False, stop=True,
                )

                # t1 = fp32(e2 + f2) — the reference's first rounding
                t1_sb = spool.tile([P, K], f32)
                nc.scalar.activation(
                    t1_sb[:],
                    e2_sb[:],
                    mybir.ActivationFunctionType.Identity,
                    bias=f2_sb[:, t : t + 1],
                    scale=1.0,
                )

                # dist = fl(t1 - 2sim) — the reference's second rounding
                dist_sb = spool.tile([P, K], f32)
                nc.vector.tensor_tensor(
                    out=dist_sb[:], in0=t1_sb[:], in1=sim_ps[:],
                    op=mybir.AluOpType.subtract,
                )
                minv = spool.tile([P, 1], f32)
                nc.vector.tensor_reduce(
                    out=minv[:], in_=dist_sb[:],
                    axis=mybir.AxisListType.X, op=mybir.AluOpType.min,
                )

                # first index where dist == min  (argmin tie-break)
                # (0-stride broadcast in_max — verified safe on HW)
                idx8 = spool.tile([P, 8], u32)
                nc.vector.max_index(
                    idx8[:], minv[:, 0:1].to_broadcast([P, 8]), dist_sb[:]
                )

                # one-hot: sign(min - dist) + 1 -> exact 1.0 at the min,
                # 0.0 elsewhere (no duplicated minima in the fp32 dist)
                sgn_sb = spool.tile([P, K], f32)
                nc.scalar.activation(
                    sgn_sb[:],
                    dist_sb[:],
                    mybir.ActivationFunctionType.Sign,
                    bias=minv[:, 0:1],
                    scale=-1.0,
                )
                enc_sb = spool.tile([P, K], f32)
                nc.scalar.activation(
                    enc_sb[:],
                    sgn_sb[:],
                    mybir.ActivationFunctionType.Identity,
                    bias=1.0,
                    scale=1.0,
                )
                nc.sync.dma_start(enc[t * P : (t + 1) * P, :], enc_sb[:])

                # gather quantized rows emb^T[idx]
                vq_sb = spool.tile([P, D], f32)
                nc.gpsimd.indirect_dma_start(
                    out=vq_sb[:],
                    out_offset=None,
                    in_=embt[:, :],
                    in_offset=IndirectOffsetOnAxis(ap=idx8[:, 0:1], axis=0),
                )
                nc.sync.dma_start(outv[t * P : (t + 1) * P, :], vq_sb[:])

    if not nc.is_finalized():
        nc.finalize()
    return nc


def _host_inputs(batch, embeddings):
    flat = np.ascontiguousarray(batch.reshape(N, D).astype(np.float32))
    emb = np.ascontiguousarray(embeddings.astype(np.float32))
    f32n = np.float32

    e2 = np.sum(emb * emb, axis=0, dtype=f32n)                  # [K]
    f2 = np.sum(flat * flat, axis=1, dtype=f32n)                # [N]

    E = (2.0 * emb).astype(f32n)
    Ehi = E.astype(npbf16)
    Er = (E - Ehi.astype(f32n)).astype(f32n)
    Emid = Er.astype(npbf16)
    Elo = (Er - Emid.astype(f32n)).astype(npbf16)

    Xhi = flat.astype(npbf16)
    Xr = (flat - Xhi.astype(f32n)).astype(f32n)
    Xmid = Xr.astype(npbf16)
    Xlo = (Xr - Xmid.astype(f32n)).astype(npbf16)

    s1 = np.concatenate([Xhi.T, Xmid.T], axis=0)                # [128, N]
    s3 = np.concatenate([Xhi.T, Xlo.T], axis=0)                 # [128, N]
    ra = np.concatenate([Ehi, Ehi], axis=0)                     # [128, 512]
    rb = np.concatenate([Emid, Emid], axis=0)                   # [128, 512]
    rc = np.concatenate([Elo, Ehi], axis=0)                     # [128, 512]

    e2bc = np.broadcast_to(e2[None, :], (P, K)).copy()          # [128, 512]
    embt = np.ascontiguousarray(emb.T)                          # [512, 64]

    in_maps = []
    for c in range(NCORES):
        sl = slice(c * RPC, (c + 1) * RPC)
        f2c = f2[sl].reshape(NTILES, P).T.astype(f32n)          # [128, 64]
        in_maps.append(
            {
                "s1": np.ascontiguousarray(s1[:, sl]),
                "s3": np.ascontiguousarray(s3[:, sl]),
                "ra": ra,
                "rb": rb,
                "rc": rc,
                "e2bc": e2bc,
                "f2a": np.ascontiguousarray(f2c),
                "embt": embt,
            }
        )
    return in_maps


def _get_nc():
    global _NC
    if _NC is None:
        _NC = _build_nc()
    return _NC


def run(batch, embeddings, trace=False):
    """Run on 8 NeuronCores. Returns ((enc, outv), BassKernelResults)."""
    nc = _get_nc()
    in_maps = _host_inputs(batch, embeddings)
    res = run_bass_kernel_spmd(nc, in_maps, list(range(NCORES)), trace=trace)
    enc = np.concatenate(
        [res.results[c]["enc"] for c in range(NCORES)], axis=0
    )
    outv = np.concatenate(
        [res.results[c]["outv"] for c in range(NCORES)], axis=0
    ).reshape(B, H, W, D)
    return (enc, outv), res


def kernel(batch, embeddings):
    (enc, outv), _ = run(batch, embeddings)
    return enc, outv


if __name__ == "__main__":
    rng = np.random.default_rng(0)
    batch = rng.standard_normal((B, H, W, D), dtype=np.float32)
    emb = rng.uniform(-0.05, 0.05, (D, K)).astype(np.float32)
    enc, outv = kernel(batch, emb)
    print(enc.shape, outv.shape, enc.sum(), np.abs(outv).mean())
